# revision 9
# baseline (speedup 1.0000x reference)
"""Trainium2 Bass kernel for a 2-layer GCN encoder + MLP head (PyG GCNConv).

Strategy (8 NeuronCores, node-parallel), v2:
  - Nodes sharded by contiguous range: core q owns rows [q*SH, (q+1)*SH).
  - conv1 linear (x @ Wc1) computed shard-local on PE (bf16, fp32 PSUM),
    z0 shards AllGather'ed to a replicated table Z0 [NP, 256] bf16.
  - Layer-2 pruning: the head only reads h2 at var_node_idx, so layer 2
    aggregates only over in-edges of the ~4k distinct variant nodes
    (owner-core sharded); no H2 AllGather, the head runs on the owner core
    and the host re-permutes the (value, b-position) pairs.
  - Layer-1 pruning: h1 is only needed at U1 = variants + their in-edge
    sources (~75% of nodes), so layer-1 tiles cover only owned U1 nodes.
  - Aggregation per dst tile: one dma_gather per (tile, half-table) fetches
    edge source rows (SWDGE queues alternate per tile to overlap descriptor
    generation), and PE matmuls accumulate st.T @ msg in PSUM where the
    norm-scaled one-hot st chunks are PREBUILT ON HOST and DMA-loaded
    (frees DVE from 2k tensor_scalar builds which contend with SWDGE).
  - dma_gather indices are int16 so the Z table is split lo/hi.
  - conv2 fused into layer-1 tile epilogue (PE transpose + matmul).
All heavy compute is bf16 with fp32 PSUM accumulation.
"""
import sys

for _p in ("/opt/trn_rl_repo",):
    if _p not in sys.path:
        sys.path.insert(0, _p)

import numpy as np
import ml_dtypes

bf16 = ml_dtypes.bfloat16

P = 128
H = 256          # gcn hidden width (fixed)
HH = 128         # head hidden width (fixed)
OH = 40          # wt_onehot + mut_onehot width (fixed)
NCORES = 8


class Cfg:
    def __init__(self, N, E, D_IN, B):
        self.N, self.E, self.D_IN, self.B = N, E, D_IN, B
        assert N % NCORES == 0
        self.SH = N // NCORES                      # real rows per shard
        shp = -(-self.SH // P) * P
        if shp == self.SH:
            shp += P                               # need >=1 dump row
        self.SHP = shp                             # padded rows per shard
        self.NP = NCORES * self.SHP                # padded global rows
        assert self.NP % 2 == 0
        self.NPH = self.NP // 2                    # half-table rows (int16 idx)
        assert self.NPH < 32768
        self.KT = -(-D_IN // P)                    # k tiles for conv1
        self.KPAD = self.KT * P
        self.MT = self.SHP // P                    # m tiles per shard
        self.B = B


REAL = Cfg(N=50000, E=800000, D_IN=1281, B=4096)


# ---------------------------------------------------------------- host prep

def _pack_idx16(seq):
    """idx sequence [n] -> wrapped-16 + replicated layout [128, n//16] int16."""
    n = seq.shape[0]
    assert n % 16 == 0
    a = seq.reshape(n // 16, 16).T.astype(np.int16)
    return np.tile(a, (8, 1))


def _pack_core(nloc, cl, ch, d_loc, srcp, nv, nph):
    """Bin-pack edges into tiles (<=128 dst nodes, <=cl*128 lo edges,
    <=ch*128 hi edges). d_loc in [0, nloc)."""
    order = np.argsort(d_loc, kind="stable")
    d_s = d_loc[order]
    counts = np.bincount(d_s, minlength=nloc)
    starts = np.zeros(nloc + 1, np.int64)
    np.cumsum(counts, out=starts[1:])
    lo_mask = srcp[order] < nph
    klo = np.zeros(nloc, np.int64)
    np.add.at(klo, d_s[lo_mask], 1)
    khi = counts - klo

    node_order = np.argsort(-counts, kind="stable")
    cap_l, cap_h = cl * P, ch * P
    tiles = []  # [n_nodes, lo_cnt, hi_cnt, node_list]
    for r in node_order:
        if counts[r] == 0 and klo[r] == 0:
            # nodes with no edges still need a slot if they are real dsts
            # (can't happen: self loops guarantee >=1 edge)
            continue
        kl, kh = klo[r], khi[r]
        placed = False
        for t in tiles:
            if t[0] < P and t[1] + kl <= cap_l and t[2] + kh <= cap_h:
                t[0] += 1
                t[1] += kl
                t[2] += kh
                t[3].append(r)
                placed = True
                break
        if not placed:
            tiles.append([1, kl, kh, [r]])
    return tiles, order, starts, lo_mask


def _build_core_arrays(T, cl, ch, tiles, order, starts, lo_mask, srcp, nv,
                       nph, scat_of, dump_row):
    """Build gidx/stt/scat arrays for one core and one layer.
    stt is the norm-scaled one-hot, host-prebuilt: [128, T*C*128] bf16."""
    C = cl + ch
    gidx_seq = np.zeros(T * C * P, np.int64)
    stt = np.zeros((P, T * C * P), bf16)
    scat = np.full((P, T), dump_row, np.int32)

    for t, tl in enumerate(tiles):
        lo_idx, lo_d, lo_n = [], [], []
        hi_idx, hi_d, hi_n = [], [], []
        for d, r in enumerate(tl[3]):
            scat[d, t] = scat_of(r)
            es = order[starts[r]:starts[r + 1]]
            lm = lo_mask[starts[r]:starts[r + 1]]
            sp = srcp[es]
            nn = nv[es]
            lo_idx.extend(sp[lm].tolist())
            lo_d.extend([d] * int(lm.sum()))
            lo_n.extend(nn[lm].tolist())
            hm = ~lm
            hi_idx.extend((sp[hm] - nph).tolist())
            hi_d.extend([d] * int(hm.sum()))
            hi_n.extend(nn[hm].tolist())
        npad_l = cl * P - len(lo_idx)
        npad_h = ch * P - len(hi_idx)
        assert npad_l >= 0 and npad_h >= 0
        seq_idx = lo_idx + [0] * npad_l + hi_idx + [0] * npad_h
        seq_d = np.asarray(lo_d + [-1] * npad_l + hi_d + [-1] * npad_h,
                           np.int64)
        seq_n = np.asarray(lo_n + [0.0] * npad_l + hi_n + [0.0] * npad_h,
                           np.float32)
        base = t * C * P
        gidx_seq[base:base + C * P] = seq_idx
        # slot j = c*128 + p  ->  stt[p, base + c*128 + d] = norm
        valid = seq_d >= 0
        jj = np.nonzero(valid)[0]
        pp = jj % P
        cc = jj // P
        stt[pp, base + cc * P + seq_d[jj]] = seq_n[jj].astype(bf16)

    cols = []
    for t in range(T):
        base = t * C * P
        cols.append(_pack_idx16(gidx_seq[base:base + cl * P]))
        cols.append(_pack_idx16(gidx_seq[base + cl * P:base + C * P]))
    gidx = np.concatenate(cols, axis=1)  # [128, T*C*8]
    return gidx, stt, scat


def _choose_caps_and_pack(nloc_list, d_loc_list, srcp_list, nv_list, nph,
                          avg_c):
    """Pick (cl, ch) caps minimizing T*(cl+ch) over all cores; return packs."""
    base = max(1, int(np.ceil(avg_c / 2)))
    cands = [(base, base), (base + 1, base + 1), (base, base + 1),
             (base + 1, base), (base + 2, base + 2)]
    best = None
    for (cl, ch) in cands:
        packs, Ts = [], []
        for q in range(NCORES):
            pk = _pack_core(nloc_list[q], cl, ch, d_loc_list[q],
                            srcp_list[q], nv_list[q], nph)
            packs.append(pk)
            Ts.append(len(pk[0]))
        T_need = max(Ts)
        cost = T_need * (cl + ch)
        if best is None or cost < best[0]:
            best = (cost, cl, ch, T_need, packs)
    _, cl, ch, T, packs = best
    return cl, ch, T, packs


def host_prep(cfg, x, wt_onehot, mut_onehot, Wc1, bc1, Wc2, bc2,
              Wh1, bh1, Wh2, bh2, Wh3, bh3, edge_index, var_node_idx):
    N, E, SH, SHP = cfg.N, cfg.E, cfg.SH, cfg.SHP
    src = np.asarray(edge_index[0], np.int64)
    dst = np.asarray(edge_index[1], np.int64)
    loop = np.arange(N, dtype=np.int64)
    src_all = np.concatenate([src, loop])
    dst_all = np.concatenate([dst, loop])
    deg = np.bincount(dst_all, minlength=N).astype(np.float32)
    dinv = np.where(deg > 0, 1.0 / np.sqrt(np.maximum(deg, 1.0)),
                    0.0).astype(np.float32)
    norm = (dinv[src_all] * dinv[dst_all]).astype(np.float32)
    SPLIT0 = 3584                       # conv1 writes z0a (4 MBS blocks) first
    NLO0 = NCORES * SPLIT0
    NHI0 = NCORES * (SHP - SPLIT0)
    assert NLO0 < 32768 and NHI0 < 32768
    _q = src_all // SH
    _r = src_all % SH
    srcp_all = np.where(_r < SPLIT0, _q * SPLIT0 + _r,
                        NLO0 + _q * (SHP - SPLIT0) + (_r - SPLIT0))

    vni = np.asarray(var_node_idx, np.int64)
    vset = np.unique(vni)
    # U1: nodes whose h1 is needed = variant nodes + sources of their in-edges
    m2 = np.isin(dst_all, vset)
    u1 = np.unique(np.concatenate([vset, src_all[m2]]))
    u1_mask = np.zeros(N, bool)
    u1_mask[u1] = True

    # ---------------- layer-1 structures (dst in owned U1)
    m1 = u1_mask[dst_all]
    d1, s1, n1 = dst_all[m1], srcp_all[m1], norm[m1]
    core1 = d1 // SH
    d1_loc, s1_l, n1_l, nloc1 = [], [], [], []
    for q in range(NCORES):
        m = core1 == q
        d1_loc.append(d1[m] - q * SH)
        s1_l.append(s1[m])
        n1_l.append(n1[m])
        nloc1.append(SH)
    avg_c1 = (m1.sum() / NCORES) / (SH * 0.754) * P / P  # rough
    avg_c1 = max(2.0, (m1.sum() / NCORES) / max(
        1, int(u1_mask.sum() / NCORES)) * 128 / 128)
    # edges per tile-of-128-dsts / 128 = chunks per tile
    avg_c1 = (m1.sum() / NCORES) / (u1_mask.sum() / NCORES / P) / P
    cl1, ch1, T1, packs1 = _choose_caps_and_pack(
        nloc1, d1_loc, s1_l, n1_l, NLO0, avg_c1)
    C1 = cl1 + ch1

    # L1 dense output layout, tile-split for overlapped AllGather:
    # tiles [0, TS1) -> z1a table, tiles [TS1, T1) -> z1b table
    T1P = T1 * P
    NP1 = NCORES * T1P
    TS1 = T1 // 2
    NLO1 = NCORES * TS1 * P
    NHI1 = NCORES * (T1 - TS1) * P
    assert NLO1 < 32768 and NHI1 < 32768
    srcp1_of = np.full(N, -1, np.int64)   # node -> dense z1 row
    for q in range(NCORES):
        tiles, _, _, _ = packs1[q]
        for t, tl in enumerate(tiles):
            for d, r in enumerate(tl[3]):
                if t < TS1:
                    srcp1_of[q * SH + r] = q * TS1 * P + t * P + d
                else:
                    srcp1_of[q * SH + r] = (NLO1 + q * (T1 - TS1) * P
                                            + (t - TS1) * P + d)

    # ---------------- layer-2 structures (dst = variant nodes, owner-shard)
    owner = vni // SH
    CAP = int(-(-max(np.bincount(owner, minlength=NCORES).max(), 1) // P) * P)
    # distinct nodes per core and local slot ids
    dist_nodes, slot_of = [], []
    for q in range(NCORES):
        vq = np.unique(vni[owner == q])
        dist_nodes.append(vq)
        sl = {int(v): i for i, v in enumerate(vq)}
        slot_of.append(sl)
    ND2 = max(len(v) for v in dist_nodes)

    m2e = np.isin(dst_all, vset)
    s2_dense = srcp1_of[src_all[m2e]]
    assert (s2_dense >= 0).all(), "L2 source not computed in L1"
    d2, s2, n2 = dst_all[m2e], s2_dense, norm[m2e]
    core2 = d2 // SH
    d2_loc, s2_l, n2_l, nloc2 = [], [], [], []
    for q in range(NCORES):
        m = core2 == q
        dd = d2[m]
        sl = slot_of[q]
        d2_loc.append(np.asarray([sl[int(v)] for v in dd], np.int64))
        s2_l.append(s2[m])
        n2_l.append(n2[m])
        nloc2.append(max(len(dist_nodes[q]), 1))
    avg_c2 = (m2e.sum() / NCORES) / max(1.0, ND2 / P) / P
    cl2, ch2, T2, packs2 = _choose_caps_and_pack(
        nloc2, d2_loc, s2_l, n2_l, NLO1, avg_c2)
    C2 = cl2 + ch2
    H2ROWS = T2 * P + P          # +dump tile row space

    # shared weights
    wc1 = np.zeros((cfg.KPAD, H), bf16)
    wc1[:cfg.D_IN] = np.asarray(Wc1, np.float32).astype(bf16)
    wc2 = np.asarray(Wc2, np.float32).astype(bf16)
    wh1 = np.zeros((3 * P, HH), bf16)
    wh1[:H + OH] = np.asarray(Wh1, np.float32).astype(bf16)
    wh2 = np.asarray(Wh2, np.float32).astype(bf16)
    wh3 = np.asarray(Wh3, np.float32).astype(bf16)
    bb1 = np.tile(np.asarray(bc1, np.float32)[None, :], (P, 1))
    bb2 = np.tile(np.asarray(bc2, np.float32)[None, :], (P, 1))
    bh1v = np.asarray(bh1, np.float32).reshape(HH, 1)
    bh2v = np.asarray(bh2, np.float32).reshape(HH // 2, 1)
    bh3v = np.asarray(bh3, np.float32).reshape(1, 1)

    x = np.asarray(x, np.float32)
    wt_b = np.asarray(wt_onehot, np.float32).astype(bf16)
    mut_b = np.asarray(mut_onehot, np.float32).astype(bf16)

    in_maps = []
    out_pos = []
    meta = dict(T1=T1, cl1=cl1, ch1=ch1, T2=T2, cl2=cl2, ch2=ch2, CAP=CAP,
                H2ROWS=H2ROWS, SPLIT0=SPLIT0, NLO0=NLO0, NHI0=NHI0,
                TS1=TS1, NLO1=NLO1, NHI1=NHI1)
    for q in range(NCORES):
        tiles, order, starts, lo_mask = packs1[q]
        gidx1, stt1, scat1 = _build_core_arrays(
            T1, cl1, ch1, tiles, order, starts, lo_mask, s1_l[q], n1_l[q],
            NLO0, scat_of=lambda r: r, dump_row=SH)
        tiles2, order2, starts2, lo_mask2 = packs2[q]
        gidx2, stt2, scat2 = _build_core_arrays(
            T2, cl2, ch2, tiles2, order2, starts2, lo_mask2, s2_l[q],
            n2_l[q], NLO1, scat_of=lambda r: 0, dump_row=T2 * P)
        # scat2 maps tile slot -> h2loc row; rebuild using slot layout
        scat2 = np.full((P, T2), T2 * P, np.int32)
        slot_row = np.full(max(len(dist_nodes[q]), 1), T2 * P, np.int64)
        for t, tl in enumerate(tiles2):
            for d, r in enumerate(tl[3]):
                scat2[d, t] = t * P + d
                slot_row[r] = t * P + d

        # per-instance rows for the head (owner order)
        inst_b = np.nonzero(owner == q)[0]          # b indices owned
        nb = len(inst_b)
        hidx = np.zeros(CAP, np.int64)
        for i, b in enumerate(inst_b):
            hidx[i] = slot_row[slot_of[q][int(vni[b])]]
        ohT = np.zeros((OH, CAP), bf16)
        ohT[:20, :nb] = wt_b[inst_b].T
        ohT[20:, :nb] = mut_b[inst_b].T
        out_pos.append(inst_b)

        xT = np.zeros((cfg.KPAD, SHP), bf16)
        xT[:cfg.D_IN, :SH] = x[q * SH:(q + 1) * SH].T.astype(bf16)
        in_maps.append(dict(
            xT=xT, gidx1=gidx1, stt1=stt1,
            gidx2=gidx2, stt2=stt2,
            hidx=_pack_idx16(hidx), ohT=np.ascontiguousarray(ohT),
            wc1=wc1, wc2=wc2, wh1=wh1, wh2=wh2, wh3=wh3,
            bb1=bb1, bb2=bb2, bh1v=bh1v, bh2v=bh2v, bh3v=bh3v,
        ))
    return in_maps, meta, out_pos


# ------------------------------------------------------------- bass program

def build_program(cfg, meta):
    import concourse.bass as bass
    import concourse.mybir as mybir
    import concourse.tile as tile
    from concourse import bacc
    from concourse.masks import make_identity

    T1, cl1, ch1 = meta["T1"], meta["cl1"], meta["ch1"]
    T2, cl2, ch2 = meta["T2"], meta["cl2"], meta["ch2"]
    CAP, H2ROWS = meta["CAP"], meta["H2ROWS"]
    C1, C2 = cl1 + ch1, cl2 + ch2
    BCH = CAP // P

    nc = bacc.Bacc("TRN2", target_bir_lowering=False, debug=False,
                   num_devices=NCORES, num_swdge_queues=4)
    f32, bfl, i16, i32 = (mybir.dt.float32, mybir.dt.bfloat16,
                          mybir.dt.int16, mybir.dt.int32)

    # I/O
    xT = nc.dram_tensor("xT", [cfg.KPAD, cfg.SHP], bfl, kind="ExternalInput")
    gidx1 = nc.dram_tensor("gidx1", [P, T1 * C1 * 8], i16,
                           kind="ExternalInput")
    stt1 = nc.dram_tensor("stt1", [P, T1 * C1 * P], bfl,
                          kind="ExternalInput")
    gidx2 = nc.dram_tensor("gidx2", [P, T2 * C2 * 8], i16,
                           kind="ExternalInput")
    stt2 = nc.dram_tensor("stt2", [P, T2 * C2 * P], bfl,
                          kind="ExternalInput")
    hidx = nc.dram_tensor("hidx", [P, CAP // 16], i16, kind="ExternalInput")
    ohT = nc.dram_tensor("ohT", [OH, CAP], bfl, kind="ExternalInput")
    wc1 = nc.dram_tensor("wc1", [cfg.KPAD, H], bfl, kind="ExternalInput")
    wc2 = nc.dram_tensor("wc2", [H, H], bfl, kind="ExternalInput")
    wh1 = nc.dram_tensor("wh1", [3 * P, HH], bfl, kind="ExternalInput")
    wh2 = nc.dram_tensor("wh2", [HH, HH // 2], bfl, kind="ExternalInput")
    wh3 = nc.dram_tensor("wh3", [HH // 2, 1], bfl, kind="ExternalInput")
    bb1 = nc.dram_tensor("bb1", [P, H], f32, kind="ExternalInput")
    bb2 = nc.dram_tensor("bb2", [P, H], f32, kind="ExternalInput")
    bh1v = nc.dram_tensor("bh1v", [HH, 1], f32, kind="ExternalInput")
    bh2v = nc.dram_tensor("bh2v", [HH // 2, 1], f32, kind="ExternalInput")
    bh3v = nc.dram_tensor("bh3v", [1, 1], f32, kind="ExternalInput")
    out = nc.dram_tensor("out", [1, CAP], f32, kind="ExternalOutput")

    SPLIT0, TS1 = meta["SPLIT0"], meta["TS1"]
    # internal DRAM (lo/hi splits so each AllGather can start early)
    z0a = nc.dram_tensor("z0a", [SPLIT0, H], bfl, kind="Internal")
    z0b = nc.dram_tensor("z0b", [cfg.SHP - SPLIT0, H], bfl, kind="Internal")
    z1a = nc.dram_tensor("z1a", [TS1 * P, H], bfl, kind="Internal")
    z1b = nc.dram_tensor("z1b", [(T1 - TS1) * P, H], bfl, kind="Internal")
    h2loc = nc.dram_tensor("h2loc", [H2ROWS, H], bfl, kind="Internal")
    Z0a = nc.dram_tensor("Z0a", [meta["NLO0"], H], bfl, kind="Internal",
                         addr_space="Shared")
    Z0b = nc.dram_tensor("Z0b", [meta["NHI0"], H], bfl, kind="Internal",
                         addr_space="Shared")
    Z1a = nc.dram_tensor("Z1a", [meta["NLO1"], H], bfl, kind="Internal",
                         addr_space="Shared")
    Z1b = nc.dram_tensor("Z1b", [meta["NHI1"], H], bfl, kind="Internal",
                         addr_space="Shared")
    rg = [list(range(NCORES))]

    with tile.TileContext(nc) as tc:
        with tc.tile_pool(name="const", bufs=1) as const:
            ident = const.tile([P, P], bfl)
            make_identity(nc, ident[:])

            def load(ap, shape, dt):
                t = const.tile(shape, dt, tag=ap.tensor.name)
                nc.sync.dma_start(t[:], ap)
                return t

            wc1_sb = load(wc1.rearrange("(t p) n -> p t n", p=P)[:],
                          [P, cfg.KT, H], bfl)
            wc2_sb = load(wc2.rearrange("(t p) n -> p t n", p=P)[:],
                          [P, 2, H], bfl)
            wh1_sb = load(wh1.rearrange("(t p) n -> p t n", p=P)[:],
                          [P, 3, HH], bfl)
            wh2_sb = load(wh2[:], [HH, HH // 2], bfl)
            wh3_sb = load(wh3[:], [HH // 2, 1], bfl)
            bb1_sb = load(bb1[:], [P, H], f32)
            bb2_sb = load(bb2[:], [P, H], f32)
            bh1_sb = load(bh1v[:], [HH, 1], f32)
            bh2_sb = load(bh2v[:], [HH // 2, 1], f32)
            bh3_sb = load(bh3v[:], [1, 1], f32)
            gidx1_sb = load(gidx1[:], [P, T1 * C1 * 8], i16)
            gidx2_sb = load(gidx2[:], [P, T2 * C2 * 8], i16)
            hidx_sb = load(hidx[:], [P, CAP // 16], i16)
            ohT_sb = load(ohT[:], [OH, CAP], bfl)


            # ---------------- phase A: conv1 linear z0 = x @ Wc1
            MBS = 7
            with tc.tile_pool(name="c1sb", bufs=3) as c1sb, \
                 tc.tile_pool(name="c1ev", bufs=3) as c1ev, \
                 tc.tile_pool(name="c1ps", bufs=MBS + 1, space="PSUM") as c1ps:
                for mb0 in range(0, cfg.MT, MBS):
                    mbn = min(MBS, cfg.MT - mb0)
                    accs = [c1ps.tile([P, H], f32, tag="convacc",
                                      name=f"convacc_{mb0}_{j}")
                            for j in range(mbn)]
                    slab = c1sb.tile([P, cfg.KT, MBS * P], bfl, tag="slab")
                    nc.sync.dma_start(
                        slab[:, :, :mbn * P],
                        xT.rearrange("(t p) n -> p t n", p=P)[
                            :, :, mb0 * P:(mb0 + mbn) * P])
                    for kt in range(cfg.KT):
                        for j in range(mbn):
                            nc.tensor.matmul(
                                accs[j][:],
                                lhsT=slab[:, kt, j * P:(j + 1) * P],
                                rhs=wc1_sb[:, kt, :],
                                start=(kt == 0), stop=(kt == cfg.KT - 1))
                    for j in range(mbn):
                        zb = c1ev.tile([P, H], bfl, tag="zev")
                        nc.vector.tensor_copy(zb[:], accs[j][:])
                        r0 = (mb0 + j) * P
                        if r0 < SPLIT0:
                            nc.sync.dma_start(z0a[r0:r0 + P, :], zb[:])
                        else:
                            nc.sync.dma_start(
                                z0b[r0 - SPLIT0:r0 - SPLIT0 + P, :], zb[:])
                    if (mb0 + mbn) * P == SPLIT0:
                        nc.gpsimd.collective_compute(
                            "AllGather", mybir.AluOpType.bypass,
                            replica_groups=rg, ins=[z0a[:]], outs=[Z0a[:]])

            nc.gpsimd.collective_compute(
                "AllGather", mybir.AluOpType.bypass, replica_groups=rg,
                ins=[z0b[:]], outs=[Z0b[:]])

            # ---------------- aggregation layers
            def agg_layer(Zlo, Zhi, T, cl, ch, gidx_sb, stt_dram, bias_sb,
                          out_fn, do_conv2, hook=None):
                C = cl + ch
                with tc.tile_pool(name="agsb", bufs=4) as agsb, \
                     tc.tile_pool(name="agst", bufs=4) as agst, \
                     tc.tile_pool(name="agps", bufs=3, space="PSUM") as agps, \
                     tc.tile_pool(name="agp2", bufs=2, space="PSUM") as agp2:
                    for t in range(T):
                        if hook is not None:
                            hook(t)
                        msg = agsb.tile([P, C, H], bfl, tag="msg")
                        off = t * C * 8
                        nc.gpsimd.dma_gather(
                            msg[:, :cl, :], Zlo, gidx_sb[:, off:off + cl * 8],
                            cl * P, cl * P, H, single_packet=False,
                            queue_num=0)
                        nc.gpsimd.dma_gather(
                            msg[:, cl:, :], Zhi,
                            gidx_sb[:, off + cl * 8:off + C * 8],
                            ch * P, ch * P, H, single_packet=False,
                            queue_num=1)
                        st = agst.tile([P, C, P], bfl, tag="st")
                        nc.sync.dma_start(
                            st[:], stt_dram[:, t * C * P:(t + 1) * C * P])
                        acc = agps.tile([P, H], f32, tag="agacc")
                        for c in range(C):
                            nc.tensor.matmul(acc[:], lhsT=st[:, c, :],
                                             rhs=msg[:, c, :],
                                             start=(c == 0), stop=(c == C - 1))
                        hf = agsb.tile([P, H], f32, tag="hf")
                        nc.vector.tensor_tensor(out=hf[:], in0=acc[:],
                                                in1=bias_sb[:],
                                                op=mybir.AluOpType.add)
                        hb = agsb.tile([P, H], bfl, tag="hb")
                        nc.scalar.activation(
                            hb[:], hf[:], mybir.ActivationFunctionType.Relu)
                        if do_conv2:
                            ht = agsb.tile([P, H], bfl, tag="ht")
                            for k in range(2):
                                pt = agp2.tile([P, P], bfl, space="PSUM",
                                               tag="pt")
                                nc.tensor.transpose(
                                    pt[:], hb[:, k * P:(k + 1) * P], ident[:])
                                nc.scalar.copy(ht[:, k * P:(k + 1) * P],
                                               pt[:])
                            pz = agp2.tile([P, H], f32, tag="pz")
                            for k in range(2):
                                nc.tensor.matmul(
                                    pz[:], lhsT=ht[:, k * P:(k + 1) * P],
                                    rhs=wc2_sb[:, k, :],
                                    start=(k == 0), stop=(k == 1))
                            res = agsb.tile([P, H], bfl, tag="res")
                            nc.vector.tensor_copy(res[:], pz[:])
                        else:
                            res = hb
                        out_fn(t, res)

            def z1_write(t, res):
                if t < TS1:
                    nc.sync.dma_start(z1a[t * P:(t + 1) * P, :], res[:])
                else:
                    t2 = t - TS1
                    nc.sync.dma_start(z1b[t2 * P:(t2 + 1) * P, :], res[:])

            def l1_hook(t):
                if t == TS1 + 3:
                    nc.gpsimd.collective_compute(
                        "AllGather", mybir.AluOpType.bypass,
                        replica_groups=rg, ins=[z1a[:]], outs=[Z1a[:]])

            agg_layer(Z0a[:], Z0b[:], T1, cl1, ch1, gidx1_sb, stt1, bb1_sb,
                      z1_write, do_conv2=True, hook=l1_hook)
            nc.gpsimd.collective_compute(
                "AllGather", mybir.AluOpType.bypass, replica_groups=rg,
                ins=[z1b[:]], outs=[Z1b[:]])

            def h2_write(t, res):
                nc.sync.dma_start(h2loc[t * P:(t + 1) * P, :], res[:])

            agg_layer(Z1a[:], Z1b[:], T2, cl2, ch2, gidx2_sb, stt2, bb2_sb,
                      h2_write, do_conv2=False)

            # ---------------- head (owner-local variants)
            with tc.tile_pool(name="hdsb", bufs=2) as hdsb, \
                 tc.tile_pool(name="hdps", bufs=1, space="PSUM") as hdps:
                g = hdsb.tile([P, BCH, H], bfl, tag="hg")
                nc.gpsimd.dma_gather(
                    g[:], h2loc[:], hidx_sb[:], CAP, CAP, H,
                    single_packet=False)
                zt0 = hdsb.tile([P, CAP], bfl, tag="zt0")
                zt1 = hdsb.tile([P, CAP], bfl, tag="zt1")
                for j in range(BCH):
                    for k in range(2):
                        pt = hdps.tile([P, P], bfl, space="PSUM", tag="hpt")
                        nc.tensor.transpose(
                            pt[:], g[:, j, k * P:(k + 1) * P], ident[:])
                        dstt = zt0 if k == 0 else zt1
                        nc.vector.tensor_copy(
                            dstt[:, j * P:(j + 1) * P], pt[:])
                ph1 = hdps.tile([P, CAP], f32, tag="ph1")
                for c0 in range(0, CAP, 512):
                    cw = min(512, CAP - c0)
                    nc.tensor.matmul(ph1[:, c0:c0 + cw],
                                     lhsT=wh1_sb[:, 0, :],
                                     rhs=zt0[:, c0:c0 + cw],
                                     start=True, stop=False)
                    nc.tensor.matmul(ph1[:, c0:c0 + cw],
                                     lhsT=wh1_sb[:, 1, :],
                                     rhs=zt1[:, c0:c0 + cw],
                                     start=False, stop=False)
                    nc.tensor.matmul(ph1[:, c0:c0 + cw],
                                     lhsT=wh1_sb[:OH, 2, :],
                                     rhs=ohT_sb[:, c0:c0 + cw],
                                     start=False, stop=True)
                a1 = hdsb.tile([P, CAP], bfl, tag="a1")
                nc.scalar.activation(a1[:], ph1[:],
                                     mybir.ActivationFunctionType.Relu,
                                     bias=bh1_sb[:])
                ph2 = hdps.tile([HH // 2, CAP], f32, tag="ph2")
                for c0 in range(0, CAP, 512):
                    cw = min(512, CAP - c0)
                    nc.tensor.matmul(ph2[:, c0:c0 + cw], lhsT=wh2_sb[:],
                                     rhs=a1[:, c0:c0 + cw],
                                     start=True, stop=True)
                a2 = hdsb.tile([HH // 2, CAP], bfl, tag="a2")
                nc.scalar.activation(a2[:], ph2[:],
                                     mybir.ActivationFunctionType.Relu,
                                     bias=bh2_sb[:])
                ph3 = hdps.tile([1, CAP], f32, tag="ph3")
                for c0 in range(0, CAP, 512):
                    cw = min(512, CAP - c0)
                    nc.tensor.matmul(ph3[:, c0:c0 + cw], lhsT=wh3_sb[:],
                                     rhs=a2[:, c0:c0 + cw],
                                     start=True, stop=True)
                osb = hdsb.tile([1, CAP], f32, tag="osb")
                nc.vector.tensor_scalar_add(osb[:], ph3[:], bh3_sb[:, :1])
                nc.sync.dma_start(out[:], osb[:])

    nc.compile()
    return nc


# ------------------------------------------------------------------ driver

_CACHE = {}


def _get_program(cfg, meta):
    key = (cfg.N, cfg.E, cfg.D_IN, cfg.B) + tuple(sorted(meta.items()))
    if key not in _CACHE:
        _CACHE[key] = build_program(cfg, meta)
    return _CACHE[key]


def kernel(**inputs):
    cfg = REAL
    in_maps, meta, out_pos = host_prep(cfg, **inputs)
    nc = _get_program(cfg, meta)
    from concourse import bass_utils
    res = bass_utils.run_bass_kernel_spmd(
        nc, in_maps, core_ids=list(range(NCORES)))
    full = np.zeros(cfg.B, np.float32)
    for q in range(NCORES):
        vals = np.asarray(res.results[q]["out"]).reshape(-1)
        full[out_pos[q]] = vals[:len(out_pos[q])]
    return full.astype(np.float32)


# revision 10
# speedup vs baseline: 1.2080x; 1.2080x over previous
"""Trainium2 Bass kernel for a 2-layer GCN encoder + MLP head (PyG GCNConv).

Strategy (8 NeuronCores, node-parallel), v2:
  - Nodes sharded by contiguous range: core q owns rows [q*SH, (q+1)*SH).
  - conv1 linear (x @ Wc1) computed shard-local on PE (bf16, fp32 PSUM),
    z0 shards AllGather'ed to a replicated table Z0 [NP, 256] bf16.
  - Layer-2 pruning: the head only reads h2 at var_node_idx, so layer 2
    aggregates only over in-edges of the ~4k distinct variant nodes
    (owner-core sharded); no H2 AllGather, the head runs on the owner core
    and the host re-permutes the (value, b-position) pairs.
  - Layer-1 pruning: h1 is only needed at U1 = variants + their in-edge
    sources (~75% of nodes), so layer-1 tiles cover only owned U1 nodes.
  - Aggregation per dst tile: one dma_gather per (tile, half-table) fetches
    edge source rows (SWDGE queues alternate per tile to overlap descriptor
    generation), and PE matmuls accumulate st.T @ msg in PSUM where the
    norm-scaled one-hot st chunks are PREBUILT ON HOST and DMA-loaded
    (frees DVE from 2k tensor_scalar builds which contend with SWDGE).
  - dma_gather indices are int16 so the Z table is split lo/hi.
  - conv2 fused into layer-1 tile epilogue (PE transpose + matmul).
All heavy compute is bf16 with fp32 PSUM accumulation.
"""
import sys

for _p in ("/opt/trn_rl_repo",):
    if _p not in sys.path:
        sys.path.insert(0, _p)

import numpy as np
import ml_dtypes

bf16 = ml_dtypes.bfloat16

P = 128
H = 256          # gcn hidden width (fixed)
HH = 128         # head hidden width (fixed)
OH = 40          # wt_onehot + mut_onehot width (fixed)
NCORES = 8


class Cfg:
    def __init__(self, N, E, D_IN, B):
        self.N, self.E, self.D_IN, self.B = N, E, D_IN, B
        assert N % NCORES == 0
        self.SH = N // NCORES                      # real rows per shard
        shp = -(-self.SH // P) * P
        if shp == self.SH:
            shp += P                               # need >=1 dump row
        self.SHP = shp                             # padded rows per shard
        self.NP = NCORES * self.SHP                # padded global rows
        assert self.NP % 2 == 0
        self.NPH = self.NP // 2                    # half-table rows (int16 idx)
        assert self.NPH < 32768
        self.KT = -(-D_IN // P)                    # k tiles for conv1
        self.KPAD = self.KT * P
        self.MT = self.SHP // P                    # m tiles per shard
        self.B = B


REAL = Cfg(N=50000, E=800000, D_IN=1281, B=4096)


# ---------------------------------------------------------------- host prep

def _pack_idx16(seq):
    """idx sequence [n] -> wrapped-16 + replicated layout [128, n//16] int16."""
    n = seq.shape[0]
    assert n % 16 == 0
    a = seq.reshape(n // 16, 16).T.astype(np.int16)
    return np.tile(a, (8, 1))


def _pack_core(nloc, cl, ch, d_loc, srcp, nv, nph):
    """Bin-pack edges into tiles (<=128 dst nodes, <=cl*128 lo edges,
    <=ch*128 hi edges). d_loc in [0, nloc)."""
    order = np.argsort(d_loc, kind="stable")
    d_s = d_loc[order]
    counts = np.bincount(d_s, minlength=nloc)
    starts = np.zeros(nloc + 1, np.int64)
    np.cumsum(counts, out=starts[1:])
    lo_mask = srcp[order] < nph
    klo = np.zeros(nloc, np.int64)
    np.add.at(klo, d_s[lo_mask], 1)
    khi = counts - klo

    node_order = np.argsort(-counts, kind="stable")
    cap_l, cap_h = cl * P, ch * P
    tiles = []  # [n_nodes, lo_cnt, hi_cnt, node_list]
    for r in node_order:
        if counts[r] == 0 and klo[r] == 0:
            # nodes with no edges still need a slot if they are real dsts
            # (can't happen: self loops guarantee >=1 edge)
            continue
        kl, kh = klo[r], khi[r]
        placed = False
        for t in tiles:
            if t[0] < P and t[1] + kl <= cap_l and t[2] + kh <= cap_h:
                t[0] += 1
                t[1] += kl
                t[2] += kh
                t[3].append(r)
                placed = True
                break
        if not placed:
            tiles.append([1, kl, kh, [r]])
    return tiles, order, starts, lo_mask


def _build_core_arrays(T, cl, ch, tiles, order, starts, lo_mask, srcp, nv,
                       nph, scat_of, dump_row):
    """Build gidx/stt/scat arrays for one core and one layer.
    stt is the norm-scaled one-hot, host-prebuilt: [128, T*C*128] bf16."""
    C = cl + ch
    gidx_seq = np.zeros(T * C * P, np.int64)
    stt = np.zeros((P, T * C * P), bf16)
    scat = np.full((P, T), dump_row, np.int32)

    for t, tl in enumerate(tiles):
        lo_idx, lo_d, lo_n = [], [], []
        hi_idx, hi_d, hi_n = [], [], []
        for d, r in enumerate(tl[3]):
            scat[d, t] = scat_of(r)
            es = order[starts[r]:starts[r + 1]]
            lm = lo_mask[starts[r]:starts[r + 1]]
            sp = srcp[es]
            nn = nv[es]
            lo_idx.extend(sp[lm].tolist())
            lo_d.extend([d] * int(lm.sum()))
            lo_n.extend(nn[lm].tolist())
            hm = ~lm
            hi_idx.extend((sp[hm] - nph).tolist())
            hi_d.extend([d] * int(hm.sum()))
            hi_n.extend(nn[hm].tolist())
        npad_l = cl * P - len(lo_idx)
        npad_h = ch * P - len(hi_idx)
        assert npad_l >= 0 and npad_h >= 0
        seq_idx = lo_idx + [0] * npad_l + hi_idx + [0] * npad_h
        seq_d = np.asarray(lo_d + [-1] * npad_l + hi_d + [-1] * npad_h,
                           np.int64)
        seq_n = np.asarray(lo_n + [0.0] * npad_l + hi_n + [0.0] * npad_h,
                           np.float32)
        base = t * C * P
        gidx_seq[base:base + C * P] = seq_idx
        # slot j = c*128 + p  ->  stt[p, base + c*128 + d] = norm
        valid = seq_d >= 0
        jj = np.nonzero(valid)[0]
        pp = jj % P
        cc = jj // P
        stt[pp, base + cc * P + seq_d[jj]] = seq_n[jj].astype(bf16)

    cols = []
    for t in range(T):
        base = t * C * P
        cols.append(_pack_idx16(gidx_seq[base:base + cl * P]))
        cols.append(_pack_idx16(gidx_seq[base + cl * P:base + C * P]))
    gidx = np.concatenate(cols, axis=1)  # [128, T*C*8]
    return gidx, stt, scat


def _choose_caps_and_pack(nloc_list, d_loc_list, srcp_list, nv_list, nph,
                          avg_c):
    """Pick (cl, ch) caps minimizing T*(cl+ch) over all cores; return packs."""
    base = max(1, int(np.ceil(avg_c / 2)))
    cands = [(base, base), (base + 1, base + 1), (base, base + 1),
             (base + 1, base), (base + 2, base + 2)]
    best = None
    for (cl, ch) in cands:
        packs, Ts = [], []
        for q in range(NCORES):
            pk = _pack_core(nloc_list[q], cl, ch, d_loc_list[q],
                            srcp_list[q], nv_list[q], nph)
            packs.append(pk)
            Ts.append(len(pk[0]))
        T_need = max(Ts)
        cost = T_need * (cl + ch)
        if best is None or cost < best[0]:
            best = (cost, cl, ch, T_need, packs)
    _, cl, ch, T, packs = best
    return cl, ch, T, packs


def host_prep(cfg, x, wt_onehot, mut_onehot, Wc1, bc1, Wc2, bc2,
              Wh1, bh1, Wh2, bh2, Wh3, bh3, edge_index, var_node_idx):
    N, E, SH, SHP = cfg.N, cfg.E, cfg.SH, cfg.SHP
    src = np.asarray(edge_index[0], np.int64)
    dst = np.asarray(edge_index[1], np.int64)
    loop = np.arange(N, dtype=np.int64)
    src_all = np.concatenate([src, loop])
    dst_all = np.concatenate([dst, loop])
    deg = np.bincount(dst_all, minlength=N).astype(np.float32)
    dinv = np.where(deg > 0, 1.0 / np.sqrt(np.maximum(deg, 1.0)),
                    0.0).astype(np.float32)
    norm = (dinv[src_all] * dinv[dst_all]).astype(np.float32)
    SPLIT0 = 3200                       # conv1 writes z0a (5 MBS blocks) first
    NLO0 = NCORES * SPLIT0
    NHI0 = NCORES * (SHP - SPLIT0)
    assert NLO0 < 32768 and NHI0 < 32768
    _q = src_all // SH
    _r = src_all % SH
    srcp_all = np.where(_r < SPLIT0, _q * SPLIT0 + _r,
                        NLO0 + _q * (SHP - SPLIT0) + (_r - SPLIT0))

    vni = np.asarray(var_node_idx, np.int64)
    vset = np.unique(vni)
    # U1: nodes whose h1 is needed = variant nodes + sources of their in-edges
    m2 = np.isin(dst_all, vset)
    u1 = np.unique(np.concatenate([vset, src_all[m2]]))
    u1_mask = np.zeros(N, bool)
    u1_mask[u1] = True

    # ---------------- layer-1 structures (dst in owned U1)
    m1 = u1_mask[dst_all]
    d1, s1, n1 = dst_all[m1], srcp_all[m1], norm[m1]
    core1 = d1 // SH
    d1_loc, s1_l, n1_l, nloc1 = [], [], [], []
    for q in range(NCORES):
        m = core1 == q
        d1_loc.append(d1[m] - q * SH)
        s1_l.append(s1[m])
        n1_l.append(n1[m])
        nloc1.append(SH)
    avg_c1 = (m1.sum() / NCORES) / (SH * 0.754) * P / P  # rough
    avg_c1 = max(2.0, (m1.sum() / NCORES) / max(
        1, int(u1_mask.sum() / NCORES)) * 128 / 128)
    # edges per tile-of-128-dsts / 128 = chunks per tile
    avg_c1 = (m1.sum() / NCORES) / (u1_mask.sum() / NCORES / P) / P
    cl1, ch1, T1, packs1 = _choose_caps_and_pack(
        nloc1, d1_loc, s1_l, n1_l, NLO0, avg_c1)
    C1 = cl1 + ch1

    # L1 dense output layout, tile-split for overlapped AllGather:
    # tiles [0, TS1) -> z1a table, tiles [TS1, T1) -> z1b table
    T1P = T1 * P
    NP1 = NCORES * T1P
    TS1 = T1 // 2
    NLO1 = NCORES * TS1 * P
    NHI1 = NCORES * (T1 - TS1) * P
    assert NLO1 < 32768 and NHI1 < 32768
    srcp1_of = np.full(N, -1, np.int64)   # node -> dense z1 row
    for q in range(NCORES):
        tiles, _, _, _ = packs1[q]
        for t, tl in enumerate(tiles):
            for d, r in enumerate(tl[3]):
                if t < TS1:
                    srcp1_of[q * SH + r] = q * TS1 * P + t * P + d
                else:
                    srcp1_of[q * SH + r] = (NLO1 + q * (T1 - TS1) * P
                                            + (t - TS1) * P + d)

    # ---------------- layer-2 structures (dst = variant nodes, owner-shard)
    owner = vni // SH
    CAP = int(-(-max(np.bincount(owner, minlength=NCORES).max(), 1) // P) * P)
    # distinct nodes per core and local slot ids
    dist_nodes, slot_of = [], []
    for q in range(NCORES):
        vq = np.unique(vni[owner == q])
        dist_nodes.append(vq)
        sl = {int(v): i for i, v in enumerate(vq)}
        slot_of.append(sl)
    ND2 = max(len(v) for v in dist_nodes)

    m2e = np.isin(dst_all, vset)
    s2_dense = srcp1_of[src_all[m2e]]
    assert (s2_dense >= 0).all(), "L2 source not computed in L1"
    d2, s2, n2 = dst_all[m2e], s2_dense, norm[m2e]
    core2 = d2 // SH
    d2_loc, s2_l, n2_l, nloc2 = [], [], [], []
    for q in range(NCORES):
        m = core2 == q
        dd = d2[m]
        sl = slot_of[q]
        d2_loc.append(np.asarray([sl[int(v)] for v in dd], np.int64))
        s2_l.append(s2[m])
        n2_l.append(n2[m])
        nloc2.append(max(len(dist_nodes[q]), 1))
    avg_c2 = (m2e.sum() / NCORES) / max(1.0, ND2 / P) / P
    cl2, ch2, T2, packs2 = _choose_caps_and_pack(
        nloc2, d2_loc, s2_l, n2_l, NLO1, avg_c2)
    C2 = cl2 + ch2
    H2ROWS = T2 * P + P          # +dump tile row space

    # shared weights
    wc1 = np.zeros((cfg.KPAD, H), bf16)
    wc1[:cfg.D_IN] = np.asarray(Wc1, np.float32).astype(bf16)
    wc2 = np.asarray(Wc2, np.float32).astype(bf16)
    wh1 = np.zeros((3 * P, HH), bf16)
    wh1[:H + OH] = np.asarray(Wh1, np.float32).astype(bf16)
    wh2 = np.asarray(Wh2, np.float32).astype(bf16)
    wh3 = np.asarray(Wh3, np.float32).astype(bf16)
    bb1 = np.tile(np.asarray(bc1, np.float32)[None, :], (P, 1))
    bb2 = np.tile(np.asarray(bc2, np.float32)[None, :], (P, 1))
    bh1v = np.asarray(bh1, np.float32).reshape(HH, 1)
    bh2v = np.asarray(bh2, np.float32).reshape(HH // 2, 1)
    bh3v = np.asarray(bh3, np.float32).reshape(1, 1)

    x = np.asarray(x, np.float32)
    wt_b = np.asarray(wt_onehot, np.float32).astype(bf16)
    mut_b = np.asarray(mut_onehot, np.float32).astype(bf16)

    in_maps = []
    out_pos = []
    meta = dict(T1=T1, cl1=cl1, ch1=ch1, T2=T2, cl2=cl2, ch2=ch2, CAP=CAP,
                H2ROWS=H2ROWS, SPLIT0=SPLIT0, NLO0=NLO0, NHI0=NHI0,
                TS1=TS1, NLO1=NLO1, NHI1=NHI1)
    for q in range(NCORES):
        tiles, order, starts, lo_mask = packs1[q]
        gidx1, stt1, scat1 = _build_core_arrays(
            T1, cl1, ch1, tiles, order, starts, lo_mask, s1_l[q], n1_l[q],
            NLO0, scat_of=lambda r: r, dump_row=SH)
        tiles2, order2, starts2, lo_mask2 = packs2[q]
        gidx2, stt2, scat2 = _build_core_arrays(
            T2, cl2, ch2, tiles2, order2, starts2, lo_mask2, s2_l[q],
            n2_l[q], NLO1, scat_of=lambda r: 0, dump_row=T2 * P)
        # scat2 maps tile slot -> h2loc row; rebuild using slot layout
        scat2 = np.full((P, T2), T2 * P, np.int32)
        slot_row = np.full(max(len(dist_nodes[q]), 1), T2 * P, np.int64)
        for t, tl in enumerate(tiles2):
            for d, r in enumerate(tl[3]):
                scat2[d, t] = t * P + d
                slot_row[r] = t * P + d

        # per-instance rows for the head (owner order)
        inst_b = np.nonzero(owner == q)[0]          # b indices owned
        nb = len(inst_b)
        hidx = np.zeros(CAP, np.int64)
        for i, b in enumerate(inst_b):
            hidx[i] = slot_row[slot_of[q][int(vni[b])]]
        ohT = np.zeros((OH, CAP), bf16)
        ohT[:20, :nb] = wt_b[inst_b].T
        ohT[20:, :nb] = mut_b[inst_b].T
        out_pos.append(inst_b)

        xT = np.zeros((cfg.KPAD, SHP), bf16)
        xT[:cfg.D_IN, :SH] = x[q * SH:(q + 1) * SH].T.astype(bf16)
        in_maps.append(dict(
            xT=xT, gidx1=gidx1, stt1=stt1,
            gidx2=gidx2, stt2=stt2,
            hidx=_pack_idx16(hidx), ohT=np.ascontiguousarray(ohT),
            wc1=wc1, wc2=wc2, wh1=wh1, wh2=wh2, wh3=wh3,
            bb1=bb1, bb2=bb2, bh1v=bh1v, bh2v=bh2v, bh3v=bh3v,
        ))
    return in_maps, meta, out_pos


# ------------------------------------------------------------- bass program

def build_program(cfg, meta):
    import concourse.bass as bass
    import concourse.mybir as mybir
    import concourse.tile as tile
    from concourse import bacc
    from concourse.masks import make_identity

    T1, cl1, ch1 = meta["T1"], meta["cl1"], meta["ch1"]
    T2, cl2, ch2 = meta["T2"], meta["cl2"], meta["ch2"]
    CAP, H2ROWS = meta["CAP"], meta["H2ROWS"]
    C1, C2 = cl1 + ch1, cl2 + ch2
    BCH = CAP // P

    nc = bacc.Bacc("TRN2", target_bir_lowering=False, debug=False,
                   num_devices=NCORES, num_swdge_queues=4)
    f32, bfl, i16, i32 = (mybir.dt.float32, mybir.dt.bfloat16,
                          mybir.dt.int16, mybir.dt.int32)

    # I/O
    xT = nc.dram_tensor("xT", [cfg.KPAD, cfg.SHP], bfl, kind="ExternalInput")
    gidx1 = nc.dram_tensor("gidx1", [P, T1 * C1 * 8], i16,
                           kind="ExternalInput")
    stt1 = nc.dram_tensor("stt1", [P, T1 * C1 * P], bfl,
                          kind="ExternalInput")
    gidx2 = nc.dram_tensor("gidx2", [P, T2 * C2 * 8], i16,
                           kind="ExternalInput")
    stt2 = nc.dram_tensor("stt2", [P, T2 * C2 * P], bfl,
                          kind="ExternalInput")
    hidx = nc.dram_tensor("hidx", [P, CAP // 16], i16, kind="ExternalInput")
    ohT = nc.dram_tensor("ohT", [OH, CAP], bfl, kind="ExternalInput")
    wc1 = nc.dram_tensor("wc1", [cfg.KPAD, H], bfl, kind="ExternalInput")
    wc2 = nc.dram_tensor("wc2", [H, H], bfl, kind="ExternalInput")
    wh1 = nc.dram_tensor("wh1", [3 * P, HH], bfl, kind="ExternalInput")
    wh2 = nc.dram_tensor("wh2", [HH, HH // 2], bfl, kind="ExternalInput")
    wh3 = nc.dram_tensor("wh3", [HH // 2, 1], bfl, kind="ExternalInput")
    bb1 = nc.dram_tensor("bb1", [P, H], f32, kind="ExternalInput")
    bb2 = nc.dram_tensor("bb2", [P, H], f32, kind="ExternalInput")
    bh1v = nc.dram_tensor("bh1v", [HH, 1], f32, kind="ExternalInput")
    bh2v = nc.dram_tensor("bh2v", [HH // 2, 1], f32, kind="ExternalInput")
    bh3v = nc.dram_tensor("bh3v", [1, 1], f32, kind="ExternalInput")
    out = nc.dram_tensor("out", [1, CAP], f32, kind="ExternalOutput")

    SPLIT0, TS1 = meta["SPLIT0"], meta["TS1"]
    # internal DRAM (lo/hi splits so each AllGather can start early)
    z0a = nc.dram_tensor("z0a", [SPLIT0, H], bfl, kind="Internal")
    z0b = nc.dram_tensor("z0b", [cfg.SHP - SPLIT0, H], bfl, kind="Internal")
    z1a = nc.dram_tensor("z1a", [TS1 * P, H], bfl, kind="Internal")
    z1b = nc.dram_tensor("z1b", [(T1 - TS1) * P, H], bfl, kind="Internal")
    h2loc = nc.dram_tensor("h2loc", [H2ROWS, H], bfl, kind="Internal")
    Z0a = nc.dram_tensor("Z0a", [meta["NLO0"], H], bfl, kind="Internal",
                         addr_space="Shared")
    Z0b = nc.dram_tensor("Z0b", [meta["NHI0"], H], bfl, kind="Internal",
                         addr_space="Shared")
    Z1a = nc.dram_tensor("Z1a", [meta["NLO1"], H], bfl, kind="Internal",
                         addr_space="Shared")
    Z1b = nc.dram_tensor("Z1b", [meta["NHI1"], H], bfl, kind="Internal",
                         addr_space="Shared")
    rg = [list(range(NCORES))]

    with tile.TileContext(nc) as tc:
        with tc.tile_pool(name="const", bufs=1) as const:
            ident = const.tile([P, P], bfl)
            make_identity(nc, ident[:])

            def load(ap, shape, dt):
                t = const.tile(shape, dt, tag=ap.tensor.name)
                nc.sync.dma_start(t[:], ap)
                return t

            wc1_sb = load(wc1.rearrange("(t p) n -> p t n", p=P)[:],
                          [P, cfg.KT, H], bfl)
            wc2_sb = load(wc2.rearrange("(t p) n -> p t n", p=P)[:],
                          [P, 2, H], bfl)
            wh1_sb = load(wh1.rearrange("(t p) n -> p t n", p=P)[:],
                          [P, 3, HH], bfl)
            wh2_sb = load(wh2[:], [HH, HH // 2], bfl)
            wh3_sb = load(wh3[:], [HH // 2, 1], bfl)
            bb1_sb = load(bb1[:], [P, H], f32)
            bb2_sb = load(bb2[:], [P, H], f32)
            bh1_sb = load(bh1v[:], [HH, 1], f32)
            bh2_sb = load(bh2v[:], [HH // 2, 1], f32)
            bh3_sb = load(bh3v[:], [1, 1], f32)
            gidx1_sb = load(gidx1[:], [P, T1 * C1 * 8], i16)
            gidx2_sb = load(gidx2[:], [P, T2 * C2 * 8], i16)
            hidx_sb = load(hidx[:], [P, CAP // 16], i16)
            ohT_sb = load(ohT[:], [OH, CAP], bfl)


            # ---------------- phase A: conv1 linear z0 = x @ Wc1
            MBS = 5
            with tc.tile_pool(name="c1sb", bufs=3) as c1sb, \
                 tc.tile_pool(name="c1ev", bufs=3) as c1ev, \
                 tc.tile_pool(name="c1ps", bufs=MBS + 1, space="PSUM") as c1ps:
                for mb0 in range(0, cfg.MT, MBS):
                    mbn = min(MBS, cfg.MT - mb0)
                    accs = [c1ps.tile([P, H], f32, tag="convacc",
                                      name=f"convacc_{mb0}_{j}")
                            for j in range(mbn)]
                    slab = c1sb.tile([P, cfg.KT, MBS * P], bfl, tag="slab")
                    nc.sync.dma_start(
                        slab[:, :, :mbn * P],
                        xT.rearrange("(t p) n -> p t n", p=P)[
                            :, :, mb0 * P:(mb0 + mbn) * P])
                    for kt in range(cfg.KT):
                        for j in range(mbn):
                            nc.tensor.matmul(
                                accs[j][:],
                                lhsT=slab[:, kt, j * P:(j + 1) * P],
                                rhs=wc1_sb[:, kt, :],
                                start=(kt == 0), stop=(kt == cfg.KT - 1))
                    for j in range(mbn):
                        zb = c1ev.tile([P, H], bfl, tag="zev")
                        nc.vector.tensor_copy(zb[:], accs[j][:])
                        r0 = (mb0 + j) * P
                        if r0 < SPLIT0:
                            nc.sync.dma_start(z0a[r0:r0 + P, :], zb[:])
                        else:
                            nc.sync.dma_start(
                                z0b[r0 - SPLIT0:r0 - SPLIT0 + P, :], zb[:])
                    if (mb0 + mbn) * P == SPLIT0:
                        nc.gpsimd.collective_compute(
                            "AllGather", mybir.AluOpType.bypass,
                            replica_groups=rg, ins=[z0a[:]], outs=[Z0a[:]])

            nc.gpsimd.collective_compute(
                "AllGather", mybir.AluOpType.bypass, replica_groups=rg,
                ins=[z0b[:]], outs=[Z0b[:]])

            # ---------------- aggregation layers
            def agg_layer(Zlo, Zhi, T, cl, ch, gidx_sb, stt_dram, bias_sb,
                          out_fn, do_conv2, hook=None):
                C = cl + ch
                with tc.tile_pool(name="agsb", bufs=4) as agsb, \
                     tc.tile_pool(name="agst", bufs=4) as agst, \
                     tc.tile_pool(name="agps", bufs=3, space="PSUM") as agps, \
                     tc.tile_pool(name="agp2", bufs=2, space="PSUM") as agp2:
                    for t in range(T):
                        if hook is not None:
                            hook(t)
                        msg = agsb.tile([P, C, H], bfl, tag="msg")
                        off = t * C * 8
                        nc.gpsimd.dma_gather(
                            msg[:, :cl, :], Zlo, gidx_sb[:, off:off + cl * 8],
                            cl * P, cl * P, H, single_packet=False,
                            queue_num=0)
                        nc.gpsimd.dma_gather(
                            msg[:, cl:, :], Zhi,
                            gidx_sb[:, off + cl * 8:off + C * 8],
                            ch * P, ch * P, H, single_packet=False,
                            queue_num=1)
                        st = agst.tile([P, C, P], bfl, tag="st")
                        nc.sync.dma_start(
                            st[:], stt_dram[:, t * C * P:(t + 1) * C * P])
                        acc = agps.tile([P, H], f32, tag="agacc")
                        for c in range(C):
                            nc.tensor.matmul(acc[:], lhsT=st[:, c, :],
                                             rhs=msg[:, c, :],
                                             start=(c == 0), stop=(c == C - 1))
                        hf = agsb.tile([P, H], f32, tag="hf")
                        nc.vector.tensor_tensor(out=hf[:], in0=acc[:],
                                                in1=bias_sb[:],
                                                op=mybir.AluOpType.add)
                        hb = agsb.tile([P, H], bfl, tag="hb")
                        nc.scalar.activation(
                            hb[:], hf[:], mybir.ActivationFunctionType.Relu)
                        if do_conv2:
                            ht = agsb.tile([P, H], bfl, tag="ht")
                            for k in range(2):
                                pt = agp2.tile([P, P], bfl, space="PSUM",
                                               tag="pt")
                                nc.tensor.transpose(
                                    pt[:], hb[:, k * P:(k + 1) * P], ident[:])
                                nc.scalar.copy(ht[:, k * P:(k + 1) * P],
                                               pt[:])
                            pz = agp2.tile([P, H], f32, tag="pz")
                            for k in range(2):
                                nc.tensor.matmul(
                                    pz[:], lhsT=ht[:, k * P:(k + 1) * P],
                                    rhs=wc2_sb[:, k, :],
                                    start=(k == 0), stop=(k == 1))
                            res = agsb.tile([P, H], bfl, tag="res")
                            nc.vector.tensor_copy(res[:], pz[:])
                        else:
                            res = hb
                        out_fn(t, res)

            def z1_write(t, res):
                if t < TS1:
                    nc.sync.dma_start(z1a[t * P:(t + 1) * P, :], res[:])
                else:
                    t2 = t - TS1
                    nc.sync.dma_start(z1b[t2 * P:(t2 + 1) * P, :], res[:])

            def l1_hook(t):
                if t == TS1 + 3:
                    nc.gpsimd.collective_compute(
                        "AllGather", mybir.AluOpType.bypass,
                        replica_groups=rg, ins=[z1a[:]], outs=[Z1a[:]])

            agg_layer(Z0a[:], Z0b[:], T1, cl1, ch1, gidx1_sb, stt1, bb1_sb,
                      z1_write, do_conv2=True, hook=l1_hook)
            nc.gpsimd.collective_compute(
                "AllGather", mybir.AluOpType.bypass, replica_groups=rg,
                ins=[z1b[:]], outs=[Z1b[:]])

            def h2_write(t, res):
                nc.sync.dma_start(h2loc[t * P:(t + 1) * P, :], res[:])

            agg_layer(Z1a[:], Z1b[:], T2, cl2, ch2, gidx2_sb, stt2, bb2_sb,
                      h2_write, do_conv2=False)

            # ---------------- head (owner-local variants)
            with tc.tile_pool(name="hdsb", bufs=2) as hdsb, \
                 tc.tile_pool(name="hdps", bufs=1, space="PSUM") as hdps:
                g = hdsb.tile([P, BCH, H], bfl, tag="hg")
                nc.gpsimd.dma_gather(
                    g[:], h2loc[:], hidx_sb[:], CAP, CAP, H,
                    single_packet=False)
                zt0 = hdsb.tile([P, CAP], bfl, tag="zt0")
                zt1 = hdsb.tile([P, CAP], bfl, tag="zt1")
                for j in range(BCH):
                    for k in range(2):
                        pt = hdps.tile([P, P], bfl, space="PSUM", tag="hpt")
                        nc.tensor.transpose(
                            pt[:], g[:, j, k * P:(k + 1) * P], ident[:])
                        dstt = zt0 if k == 0 else zt1
                        nc.vector.tensor_copy(
                            dstt[:, j * P:(j + 1) * P], pt[:])
                ph1 = hdps.tile([P, CAP], f32, tag="ph1")
                for c0 in range(0, CAP, 512):
                    cw = min(512, CAP - c0)
                    nc.tensor.matmul(ph1[:, c0:c0 + cw],
                                     lhsT=wh1_sb[:, 0, :],
                                     rhs=zt0[:, c0:c0 + cw],
                                     start=True, stop=False)
                    nc.tensor.matmul(ph1[:, c0:c0 + cw],
                                     lhsT=wh1_sb[:, 1, :],
                                     rhs=zt1[:, c0:c0 + cw],
                                     start=False, stop=False)
                    nc.tensor.matmul(ph1[:, c0:c0 + cw],
                                     lhsT=wh1_sb[:OH, 2, :],
                                     rhs=ohT_sb[:, c0:c0 + cw],
                                     start=False, stop=True)
                a1 = hdsb.tile([P, CAP], bfl, tag="a1")
                nc.scalar.activation(a1[:], ph1[:],
                                     mybir.ActivationFunctionType.Relu,
                                     bias=bh1_sb[:])
                ph2 = hdps.tile([HH // 2, CAP], f32, tag="ph2")
                for c0 in range(0, CAP, 512):
                    cw = min(512, CAP - c0)
                    nc.tensor.matmul(ph2[:, c0:c0 + cw], lhsT=wh2_sb[:],
                                     rhs=a1[:, c0:c0 + cw],
                                     start=True, stop=True)
                a2 = hdsb.tile([HH // 2, CAP], bfl, tag="a2")
                nc.scalar.activation(a2[:], ph2[:],
                                     mybir.ActivationFunctionType.Relu,
                                     bias=bh2_sb[:])
                ph3 = hdps.tile([1, CAP], f32, tag="ph3")
                for c0 in range(0, CAP, 512):
                    cw = min(512, CAP - c0)
                    nc.tensor.matmul(ph3[:, c0:c0 + cw], lhsT=wh3_sb[:],
                                     rhs=a2[:, c0:c0 + cw],
                                     start=True, stop=True)
                osb = hdsb.tile([1, CAP], f32, tag="osb")
                nc.vector.tensor_scalar_add(osb[:], ph3[:], bh3_sb[:, :1])
                nc.sync.dma_start(out[:], osb[:])

    nc.compile()
    return nc


# ------------------------------------------------------------------ driver

_CACHE = {}


def _get_program(cfg, meta):
    key = (cfg.N, cfg.E, cfg.D_IN, cfg.B) + tuple(sorted(meta.items()))
    if key not in _CACHE:
        _CACHE[key] = build_program(cfg, meta)
    return _CACHE[key]


def kernel(**inputs):
    cfg = REAL
    in_maps, meta, out_pos = host_prep(cfg, **inputs)
    nc = _get_program(cfg, meta)
    from concourse import bass_utils
    res = bass_utils.run_bass_kernel_spmd(
        nc, in_maps, core_ids=list(range(NCORES)))
    full = np.zeros(cfg.B, np.float32)
    for q in range(NCORES):
        vals = np.asarray(res.results[q]["out"]).reshape(-1)
        full[out_pos[q]] = vals[:len(out_pos[q])]
    return full.astype(np.float32)


# revision 11
# speedup vs baseline: 1.2192x; 1.0093x over previous
"""Trainium2 Bass kernel for a 2-layer GCN encoder + MLP head (PyG GCNConv).

Strategy (8 NeuronCores, node-parallel), v2:
  - Nodes sharded by contiguous range: core q owns rows [q*SH, (q+1)*SH).
  - conv1 linear (x @ Wc1) computed shard-local on PE (bf16, fp32 PSUM),
    z0 shards AllGather'ed to a replicated table Z0 [NP, 256] bf16.
  - Layer-2 pruning: the head only reads h2 at var_node_idx, so layer 2
    aggregates only over in-edges of the ~4k distinct variant nodes
    (owner-core sharded); no H2 AllGather, the head runs on the owner core
    and the host re-permutes the (value, b-position) pairs.
  - Layer-1 pruning: h1 is only needed at U1 = variants + their in-edge
    sources (~75% of nodes), so layer-1 tiles cover only owned U1 nodes.
  - Aggregation per dst tile: one dma_gather per (tile, half-table) fetches
    edge source rows (SWDGE queues alternate per tile to overlap descriptor
    generation), and PE matmuls accumulate st.T @ msg in PSUM where the
    norm-scaled one-hot st chunks are PREBUILT ON HOST and DMA-loaded
    (frees DVE from 2k tensor_scalar builds which contend with SWDGE).
  - dma_gather indices are int16 so the Z table is split lo/hi.
  - conv2 fused into layer-1 tile epilogue (PE transpose + matmul).
All heavy compute is bf16 with fp32 PSUM accumulation.
"""
import sys

for _p in ("/opt/trn_rl_repo",):
    if _p not in sys.path:
        sys.path.insert(0, _p)

import numpy as np
import ml_dtypes

bf16 = ml_dtypes.bfloat16

P = 128
H = 256          # gcn hidden width (fixed)
HH = 128         # head hidden width (fixed)
OH = 40          # wt_onehot + mut_onehot width (fixed)
NCORES = 8


class Cfg:
    def __init__(self, N, E, D_IN, B):
        self.N, self.E, self.D_IN, self.B = N, E, D_IN, B
        assert N % NCORES == 0
        self.SH = N // NCORES                      # real rows per shard
        shp = -(-self.SH // P) * P
        if shp == self.SH:
            shp += P                               # need >=1 dump row
        self.SHP = shp                             # padded rows per shard
        self.NP = NCORES * self.SHP                # padded global rows
        assert self.NP % 2 == 0
        self.NPH = self.NP // 2                    # half-table rows (int16 idx)
        assert self.NPH < 32768
        self.KT = -(-D_IN // P)                    # k tiles for conv1
        self.KPAD = self.KT * P
        self.MT = self.SHP // P                    # m tiles per shard
        self.B = B


REAL = Cfg(N=50000, E=800000, D_IN=1281, B=4096)


# ---------------------------------------------------------------- host prep

def _pack_idx16(seq):
    """idx sequence [n] -> wrapped-16 + replicated layout [128, n//16] int16."""
    n = seq.shape[0]
    assert n % 16 == 0
    a = seq.reshape(n // 16, 16).T.astype(np.int16)
    return np.tile(a, (8, 1))


def _pack_core(nloc, cl, ch, d_loc, srcp, nv, nph):
    """Bin-pack edges into tiles (<=128 dst nodes, <=cl*128 lo edges,
    <=ch*128 hi edges). d_loc in [0, nloc)."""
    order = np.argsort(d_loc, kind="stable")
    d_s = d_loc[order]
    counts = np.bincount(d_s, minlength=nloc)
    starts = np.zeros(nloc + 1, np.int64)
    np.cumsum(counts, out=starts[1:])
    lo_mask = srcp[order] < nph
    klo = np.zeros(nloc, np.int64)
    np.add.at(klo, d_s[lo_mask], 1)
    khi = counts - klo

    node_order = np.argsort(-counts, kind="stable")
    cap_l, cap_h = cl * P, ch * P
    tiles = []  # [n_nodes, lo_cnt, hi_cnt, node_list]
    for r in node_order:
        if counts[r] == 0 and klo[r] == 0:
            # nodes with no edges still need a slot if they are real dsts
            # (can't happen: self loops guarantee >=1 edge)
            continue
        kl, kh = klo[r], khi[r]
        placed = False
        for t in tiles:
            if t[0] < P and t[1] + kl <= cap_l and t[2] + kh <= cap_h:
                t[0] += 1
                t[1] += kl
                t[2] += kh
                t[3].append(r)
                placed = True
                break
        if not placed:
            tiles.append([1, kl, kh, [r]])
    return tiles, order, starts, lo_mask


def _build_core_arrays(T, cl, ch, tiles, order, starts, lo_mask, srcp, nv,
                       nph, scat_of, dump_row):
    """Build gidx/stt/scat arrays for one core and one layer.
    stt is the norm-scaled one-hot, host-prebuilt: [128, T*C*128] bf16."""
    C = cl + ch
    gidx_seq = np.zeros(T * C * P, np.int64)
    stt = np.zeros((P, T * C * P), bf16)
    scat = np.full((P, T), dump_row, np.int32)

    for t, tl in enumerate(tiles):
        lo_idx, lo_d, lo_n = [], [], []
        hi_idx, hi_d, hi_n = [], [], []
        for d, r in enumerate(tl[3]):
            scat[d, t] = scat_of(r)
            es = order[starts[r]:starts[r + 1]]
            lm = lo_mask[starts[r]:starts[r + 1]]
            sp = srcp[es]
            nn = nv[es]
            lo_idx.extend(sp[lm].tolist())
            lo_d.extend([d] * int(lm.sum()))
            lo_n.extend(nn[lm].tolist())
            hm = ~lm
            hi_idx.extend((sp[hm] - nph).tolist())
            hi_d.extend([d] * int(hm.sum()))
            hi_n.extend(nn[hm].tolist())
        npad_l = cl * P - len(lo_idx)
        npad_h = ch * P - len(hi_idx)
        assert npad_l >= 0 and npad_h >= 0
        seq_idx = lo_idx + [0] * npad_l + hi_idx + [0] * npad_h
        seq_d = np.asarray(lo_d + [-1] * npad_l + hi_d + [-1] * npad_h,
                           np.int64)
        seq_n = np.asarray(lo_n + [0.0] * npad_l + hi_n + [0.0] * npad_h,
                           np.float32)
        base = t * C * P
        gidx_seq[base:base + C * P] = seq_idx
        # slot j = c*128 + p  ->  stt[p, base + c*128 + d] = norm
        valid = seq_d >= 0
        jj = np.nonzero(valid)[0]
        pp = jj % P
        cc = jj // P
        stt[pp, base + cc * P + seq_d[jj]] = seq_n[jj].astype(bf16)

    cols = []
    for t in range(T):
        base = t * C * P
        cols.append(_pack_idx16(gidx_seq[base:base + cl * P]))
        cols.append(_pack_idx16(gidx_seq[base + cl * P:base + C * P]))
    gidx = np.concatenate(cols, axis=1)  # [128, T*C*8]
    return gidx, stt, scat


def _choose_caps_and_pack(nloc_list, d_loc_list, srcp_list, nv_list, nph,
                          avg_c):
    """Pick (cl, ch) caps minimizing T*(cl+ch) over all cores; return packs."""
    base = max(1, int(np.ceil(avg_c / 2)))
    cands = [(base, base), (base + 1, base + 1), (base, base + 1),
             (base + 1, base), (base + 2, base + 2)]
    best = None
    for (cl, ch) in cands:
        packs, Ts = [], []
        for q in range(NCORES):
            pk = _pack_core(nloc_list[q], cl, ch, d_loc_list[q],
                            srcp_list[q], nv_list[q], nph)
            packs.append(pk)
            Ts.append(len(pk[0]))
        T_need = max(Ts)
        cost = T_need * (cl + ch)
        if best is None or cost < best[0]:
            best = (cost, cl, ch, T_need, packs)
    _, cl, ch, T, packs = best
    return cl, ch, T, packs


def host_prep(cfg, x, wt_onehot, mut_onehot, Wc1, bc1, Wc2, bc2,
              Wh1, bh1, Wh2, bh2, Wh3, bh3, edge_index, var_node_idx):
    N, E, SH, SHP = cfg.N, cfg.E, cfg.SH, cfg.SHP
    src = np.asarray(edge_index[0], np.int64)
    dst = np.asarray(edge_index[1], np.int64)
    loop = np.arange(N, dtype=np.int64)
    src_all = np.concatenate([src, loop])
    dst_all = np.concatenate([dst, loop])
    deg = np.bincount(dst_all, minlength=N).astype(np.float32)
    dinv = np.where(deg > 0, 1.0 / np.sqrt(np.maximum(deg, 1.0)),
                    0.0).astype(np.float32)
    norm = (dinv[src_all] * dinv[dst_all]).astype(np.float32)
    SPLIT0 = 3200                       # conv1 writes z0a (5 MBS blocks) first
    NLO0 = NCORES * SPLIT0
    NHI0 = NCORES * (SHP - SPLIT0)
    assert NLO0 < 32768 and NHI0 < 32768
    _q = src_all // SH
    _r = src_all % SH
    srcp_all = np.where(_r < SPLIT0, _q * SPLIT0 + _r,
                        NLO0 + _q * (SHP - SPLIT0) + (_r - SPLIT0))

    vni = np.asarray(var_node_idx, np.int64)
    vset = np.unique(vni)
    # U1: nodes whose h1 is needed = variant nodes + sources of their in-edges
    m2 = np.isin(dst_all, vset)
    u1 = np.unique(np.concatenate([vset, src_all[m2]]))
    u1_mask = np.zeros(N, bool)
    u1_mask[u1] = True

    # ---------------- layer-1 structures (dst in owned U1)
    m1 = u1_mask[dst_all]
    d1, s1, n1 = dst_all[m1], srcp_all[m1], norm[m1]
    core1 = d1 // SH
    d1_loc, s1_l, n1_l, nloc1 = [], [], [], []
    for q in range(NCORES):
        m = core1 == q
        d1_loc.append(d1[m] - q * SH)
        s1_l.append(s1[m])
        n1_l.append(n1[m])
        nloc1.append(SH)
    avg_c1 = (m1.sum() / NCORES) / (SH * 0.754) * P / P  # rough
    avg_c1 = max(2.0, (m1.sum() / NCORES) / max(
        1, int(u1_mask.sum() / NCORES)) * 128 / 128)
    # edges per tile-of-128-dsts / 128 = chunks per tile
    avg_c1 = (m1.sum() / NCORES) / (u1_mask.sum() / NCORES / P) / P
    cl1, ch1, T1, packs1 = _choose_caps_and_pack(
        nloc1, d1_loc, s1_l, n1_l, NLO0, avg_c1)
    C1 = cl1 + ch1

    # L1 dense output layout, tile-split for overlapped AllGather:
    # tiles [0, TS1) -> z1a table, tiles [TS1, T1) -> z1b table
    T1P = T1 * P
    NP1 = NCORES * T1P
    TS1 = T1 // 2
    NLO1 = NCORES * TS1 * P
    NHI1 = NCORES * (T1 - TS1) * P
    assert NLO1 < 32768 and NHI1 < 32768
    srcp1_of = np.full(N, -1, np.int64)   # node -> dense z1 row
    for q in range(NCORES):
        tiles, _, _, _ = packs1[q]
        for t, tl in enumerate(tiles):
            for d, r in enumerate(tl[3]):
                if t < TS1:
                    srcp1_of[q * SH + r] = q * TS1 * P + t * P + d
                else:
                    srcp1_of[q * SH + r] = (NLO1 + q * (T1 - TS1) * P
                                            + (t - TS1) * P + d)

    # ---------------- layer-2 structures (dst = variant nodes, owner-shard)
    owner = vni // SH
    CAP = int(-(-max(np.bincount(owner, minlength=NCORES).max(), 1) // P) * P)
    # distinct nodes per core and local slot ids
    dist_nodes, slot_of = [], []
    for q in range(NCORES):
        vq = np.unique(vni[owner == q])
        dist_nodes.append(vq)
        sl = {int(v): i for i, v in enumerate(vq)}
        slot_of.append(sl)
    ND2 = max(len(v) for v in dist_nodes)

    m2e = np.isin(dst_all, vset)
    s2_dense = srcp1_of[src_all[m2e]]
    assert (s2_dense >= 0).all(), "L2 source not computed in L1"
    d2, s2, n2 = dst_all[m2e], s2_dense, norm[m2e]
    core2 = d2 // SH
    d2_loc, s2_l, n2_l, nloc2 = [], [], [], []
    for q in range(NCORES):
        m = core2 == q
        dd = d2[m]
        sl = slot_of[q]
        d2_loc.append(np.asarray([sl[int(v)] for v in dd], np.int64))
        s2_l.append(s2[m])
        n2_l.append(n2[m])
        nloc2.append(max(len(dist_nodes[q]), 1))
    avg_c2 = (m2e.sum() / NCORES) / max(1.0, ND2 / P) / P
    cl2, ch2, T2, packs2 = _choose_caps_and_pack(
        nloc2, d2_loc, s2_l, n2_l, NLO1, avg_c2)
    C2 = cl2 + ch2
    H2ROWS = T2 * P + P          # +dump tile row space

    # shared weights
    wc1 = np.zeros((cfg.KPAD, H), bf16)
    wc1[:cfg.D_IN] = np.asarray(Wc1, np.float32).astype(bf16)
    wc2 = np.asarray(Wc2, np.float32).astype(bf16)
    wh1 = np.zeros((3 * P, HH), bf16)
    wh1[:H + OH] = np.asarray(Wh1, np.float32).astype(bf16)
    wh2 = np.asarray(Wh2, np.float32).astype(bf16)
    wh3 = np.asarray(Wh3, np.float32).astype(bf16)
    bb1 = np.tile(np.asarray(bc1, np.float32)[None, :], (P, 1))
    bb2 = np.tile(np.asarray(bc2, np.float32)[None, :], (P, 1))
    bh1v = np.asarray(bh1, np.float32).reshape(HH, 1)
    bh2v = np.asarray(bh2, np.float32).reshape(HH // 2, 1)
    bh3v = np.asarray(bh3, np.float32).reshape(1, 1)

    x = np.asarray(x, np.float32)
    wt_b = np.asarray(wt_onehot, np.float32).astype(bf16)
    mut_b = np.asarray(mut_onehot, np.float32).astype(bf16)

    in_maps = []
    out_pos = []
    meta = dict(T1=T1, cl1=cl1, ch1=ch1, T2=T2, cl2=cl2, ch2=ch2, CAP=CAP,
                H2ROWS=H2ROWS, SPLIT0=SPLIT0, NLO0=NLO0, NHI0=NHI0,
                TS1=TS1, NLO1=NLO1, NHI1=NHI1)
    for q in range(NCORES):
        tiles, order, starts, lo_mask = packs1[q]
        gidx1, stt1, scat1 = _build_core_arrays(
            T1, cl1, ch1, tiles, order, starts, lo_mask, s1_l[q], n1_l[q],
            NLO0, scat_of=lambda r: r, dump_row=SH)
        tiles2, order2, starts2, lo_mask2 = packs2[q]
        gidx2, stt2, scat2 = _build_core_arrays(
            T2, cl2, ch2, tiles2, order2, starts2, lo_mask2, s2_l[q],
            n2_l[q], NLO1, scat_of=lambda r: 0, dump_row=T2 * P)
        # scat2 maps tile slot -> h2loc row; rebuild using slot layout
        scat2 = np.full((P, T2), T2 * P, np.int32)
        slot_row = np.full(max(len(dist_nodes[q]), 1), T2 * P, np.int64)
        for t, tl in enumerate(tiles2):
            for d, r in enumerate(tl[3]):
                scat2[d, t] = t * P + d
                slot_row[r] = t * P + d

        # per-instance rows for the head (owner order)
        inst_b = np.nonzero(owner == q)[0]          # b indices owned
        nb = len(inst_b)
        hidx = np.zeros(CAP, np.int64)
        for i, b in enumerate(inst_b):
            hidx[i] = slot_row[slot_of[q][int(vni[b])]]
        ohT = np.zeros((OH, CAP), bf16)
        ohT[:20, :nb] = wt_b[inst_b].T
        ohT[20:, :nb] = mut_b[inst_b].T
        out_pos.append(inst_b)

        xT = np.zeros((cfg.KPAD, SHP), bf16)
        xT[:cfg.D_IN, :SH] = x[q * SH:(q + 1) * SH].T.astype(bf16)
        in_maps.append(dict(
            xT=xT, gidx1=gidx1, stt1=stt1,
            gidx2=gidx2, stt2=stt2,
            hidx=_pack_idx16(hidx), ohT=np.ascontiguousarray(ohT),
            wc1=wc1, wc2=wc2, wh1=wh1, wh2=wh2, wh3=wh3,
            bb1=bb1, bb2=bb2, bh1v=bh1v, bh2v=bh2v, bh3v=bh3v,
        ))
    return in_maps, meta, out_pos


# ------------------------------------------------------------- bass program

def build_program(cfg, meta):
    import concourse.bass as bass
    import concourse.mybir as mybir
    import concourse.tile as tile
    from concourse import bacc
    from concourse.masks import make_identity

    T1, cl1, ch1 = meta["T1"], meta["cl1"], meta["ch1"]
    T2, cl2, ch2 = meta["T2"], meta["cl2"], meta["ch2"]
    CAP, H2ROWS = meta["CAP"], meta["H2ROWS"]
    C1, C2 = cl1 + ch1, cl2 + ch2
    BCH = CAP // P

    nc = bacc.Bacc("TRN2", target_bir_lowering=False, debug=False,
                   num_devices=NCORES, num_swdge_queues=4)
    f32, bfl, i16, i32 = (mybir.dt.float32, mybir.dt.bfloat16,
                          mybir.dt.int16, mybir.dt.int32)

    # I/O
    xT = nc.dram_tensor("xT", [cfg.KPAD, cfg.SHP], bfl, kind="ExternalInput")
    gidx1 = nc.dram_tensor("gidx1", [P, T1 * C1 * 8], i16,
                           kind="ExternalInput")
    stt1 = nc.dram_tensor("stt1", [P, T1 * C1 * P], bfl,
                          kind="ExternalInput")
    gidx2 = nc.dram_tensor("gidx2", [P, T2 * C2 * 8], i16,
                           kind="ExternalInput")
    stt2 = nc.dram_tensor("stt2", [P, T2 * C2 * P], bfl,
                          kind="ExternalInput")
    hidx = nc.dram_tensor("hidx", [P, CAP // 16], i16, kind="ExternalInput")
    ohT = nc.dram_tensor("ohT", [OH, CAP], bfl, kind="ExternalInput")
    wc1 = nc.dram_tensor("wc1", [cfg.KPAD, H], bfl, kind="ExternalInput")
    wc2 = nc.dram_tensor("wc2", [H, H], bfl, kind="ExternalInput")
    wh1 = nc.dram_tensor("wh1", [3 * P, HH], bfl, kind="ExternalInput")
    wh2 = nc.dram_tensor("wh2", [HH, HH // 2], bfl, kind="ExternalInput")
    wh3 = nc.dram_tensor("wh3", [HH // 2, 1], bfl, kind="ExternalInput")
    bb1 = nc.dram_tensor("bb1", [P, H], f32, kind="ExternalInput")
    bb2 = nc.dram_tensor("bb2", [P, H], f32, kind="ExternalInput")
    bh1v = nc.dram_tensor("bh1v", [HH, 1], f32, kind="ExternalInput")
    bh2v = nc.dram_tensor("bh2v", [HH // 2, 1], f32, kind="ExternalInput")
    bh3v = nc.dram_tensor("bh3v", [1, 1], f32, kind="ExternalInput")
    out = nc.dram_tensor("out", [1, CAP], f32, kind="ExternalOutput")

    SPLIT0, TS1 = meta["SPLIT0"], meta["TS1"]
    # internal DRAM (lo/hi splits so each AllGather can start early)
    z0a = nc.dram_tensor("z0a", [SPLIT0, H], bfl, kind="Internal")
    z0b = nc.dram_tensor("z0b", [cfg.SHP - SPLIT0, H], bfl, kind="Internal")
    z1a = nc.dram_tensor("z1a", [TS1 * P, H], bfl, kind="Internal")
    z1b = nc.dram_tensor("z1b", [(T1 - TS1) * P, H], bfl, kind="Internal")
    h2loc = nc.dram_tensor("h2loc", [H2ROWS, H], bfl, kind="Internal")
    Z0a = nc.dram_tensor("Z0a", [meta["NLO0"], H], bfl, kind="Internal",
                         addr_space="Shared")
    Z0b = nc.dram_tensor("Z0b", [meta["NHI0"], H], bfl, kind="Internal",
                         addr_space="Shared")
    Z1a = nc.dram_tensor("Z1a", [meta["NLO1"], H], bfl, kind="Internal",
                         addr_space="Shared")
    Z1b = nc.dram_tensor("Z1b", [meta["NHI1"], H], bfl, kind="Internal",
                         addr_space="Shared")
    rg = [list(range(NCORES))]

    with tile.TileContext(nc) as tc:
        with tc.tile_pool(name="const", bufs=1) as const:
            ident = const.tile([P, P], bfl)
            make_identity(nc, ident[:])

            def load(ap, shape, dt):
                t = const.tile(shape, dt, tag=ap.tensor.name)
                nc.sync.dma_start(t[:], ap)
                return t

            wc1_sb = load(wc1.rearrange("(t p) n -> p t n", p=P)[:],
                          [P, cfg.KT, H], bfl)
            wc2_sb = load(wc2.rearrange("(t p) n -> p t n", p=P)[:],
                          [P, 2, H], bfl)
            wh1_sb = load(wh1.rearrange("(t p) n -> p t n", p=P)[:],
                          [P, 3, HH], bfl)
            wh2_sb = load(wh2[:], [HH, HH // 2], bfl)
            wh3_sb = load(wh3[:], [HH // 2, 1], bfl)
            bb1_sb = load(bb1[:], [P, H], f32)
            bb2_sb = load(bb2[:], [P, H], f32)
            bh1_sb = load(bh1v[:], [HH, 1], f32)
            bh2_sb = load(bh2v[:], [HH // 2, 1], f32)
            bh3_sb = load(bh3v[:], [1, 1], f32)
            gidx1_sb = load(gidx1[:], [P, T1 * C1 * 8], i16)
            gidx2_sb = load(gidx2[:], [P, T2 * C2 * 8], i16)
            hidx_sb = load(hidx[:], [P, CAP // 16], i16)
            ohT_sb = load(ohT[:], [OH, CAP], bfl)


            # ---------------- phase A: conv1 linear z0 = x @ Wc1
            MBS = 5
            with tc.tile_pool(name="c1sb", bufs=3) as c1sb, \
                 tc.tile_pool(name="c1ev", bufs=3) as c1ev, \
                 tc.tile_pool(name="c1ps", bufs=MBS + 1, space="PSUM") as c1ps:
                for mb0 in range(0, cfg.MT, MBS):
                    mbn = min(MBS, cfg.MT - mb0)
                    accs = [c1ps.tile([P, H], f32, tag="convacc",
                                      name=f"convacc_{mb0}_{j}")
                            for j in range(mbn)]
                    slab = c1sb.tile([P, cfg.KT, MBS * P], bfl, tag="slab")
                    nc.sync.dma_start(
                        slab[:, :, :mbn * P],
                        xT.rearrange("(t p) n -> p t n", p=P)[
                            :, :, mb0 * P:(mb0 + mbn) * P])
                    for kt in range(cfg.KT):
                        for j in range(mbn):
                            nc.tensor.matmul(
                                accs[j][:],
                                lhsT=slab[:, kt, j * P:(j + 1) * P],
                                rhs=wc1_sb[:, kt, :],
                                start=(kt == 0), stop=(kt == cfg.KT - 1))
                    for j in range(mbn):
                        zb = c1ev.tile([P, H], bfl, tag="zev")
                        nc.vector.tensor_copy(zb[:], accs[j][:])
                        r0 = (mb0 + j) * P
                        if r0 < SPLIT0:
                            nc.sync.dma_start(z0a[r0:r0 + P, :], zb[:])
                        else:
                            nc.sync.dma_start(
                                z0b[r0 - SPLIT0:r0 - SPLIT0 + P, :], zb[:])
                    if (mb0 + mbn) * P == SPLIT0:
                        nc.gpsimd.collective_compute(
                            "AllGather", mybir.AluOpType.bypass,
                            replica_groups=rg, ins=[z0a[:]], outs=[Z0a[:]])

            nc.gpsimd.collective_compute(
                "AllGather", mybir.AluOpType.bypass, replica_groups=rg,
                ins=[z0b[:]], outs=[Z0b[:]])

            # ---------------- aggregation layers
            def agg_layer(Zlo, Zhi, T, cl, ch, gidx_sb, stt_dram, bias_sb,
                          out_fn, do_conv2, hook=None):
                C = cl + ch
                with tc.tile_pool(name="agsb", bufs=4) as agsb, \
                     tc.tile_pool(name="agst", bufs=4) as agst, \
                     tc.tile_pool(name="agps", bufs=3, space="PSUM") as agps, \
                     tc.tile_pool(name="agp2", bufs=2, space="PSUM") as agp2:
                    cl0 = (cl + 1) // 2
                    ch0 = (ch + 1) // 2
                    for t in range(T):
                        if hook is not None:
                            hook(t)
                        msg = agsb.tile([P, C, H], bfl, tag="msg")
                        off = t * C * 8
                        nc.gpsimd.dma_gather(
                            msg[:, :cl0, :], Zlo,
                            gidx_sb[:, off:off + cl0 * 8],
                            cl0 * P, cl0 * P, H, single_packet=False,
                            queue_num=0)
                        nc.gpsimd.dma_gather(
                            msg[:, cl0:cl, :], Zlo,
                            gidx_sb[:, off + cl0 * 8:off + cl * 8],
                            (cl - cl0) * P, (cl - cl0) * P, H,
                            single_packet=False, queue_num=2)
                        nc.gpsimd.dma_gather(
                            msg[:, cl:cl + ch0, :], Zhi,
                            gidx_sb[:, off + cl * 8:off + (cl + ch0) * 8],
                            ch0 * P, ch0 * P, H, single_packet=False,
                            queue_num=1)
                        nc.gpsimd.dma_gather(
                            msg[:, cl + ch0:, :], Zhi,
                            gidx_sb[:, off + (cl + ch0) * 8:off + C * 8],
                            (ch - ch0) * P, (ch - ch0) * P, H,
                            single_packet=False, queue_num=3)
                        st = agst.tile([P, C, P], bfl, tag="st")
                        nc.sync.dma_start(
                            st[:], stt_dram[:, t * C * P:(t + 1) * C * P])
                        acc = agps.tile([P, H], f32, tag="agacc")
                        for c in range(C):
                            nc.tensor.matmul(acc[:], lhsT=st[:, c, :],
                                             rhs=msg[:, c, :],
                                             start=(c == 0), stop=(c == C - 1))
                        hf = agsb.tile([P, H], f32, tag="hf")
                        nc.vector.tensor_tensor(out=hf[:], in0=acc[:],
                                                in1=bias_sb[:],
                                                op=mybir.AluOpType.add)
                        hb = agsb.tile([P, H], bfl, tag="hb")
                        nc.scalar.activation(
                            hb[:], hf[:], mybir.ActivationFunctionType.Relu)
                        if do_conv2:
                            ht = agsb.tile([P, H], bfl, tag="ht")
                            for k in range(2):
                                pt = agp2.tile([P, P], bfl, space="PSUM",
                                               tag="pt")
                                nc.tensor.transpose(
                                    pt[:], hb[:, k * P:(k + 1) * P], ident[:])
                                nc.scalar.copy(ht[:, k * P:(k + 1) * P],
                                               pt[:])
                            pz = agp2.tile([P, H], f32, tag="pz")
                            for k in range(2):
                                nc.tensor.matmul(
                                    pz[:], lhsT=ht[:, k * P:(k + 1) * P],
                                    rhs=wc2_sb[:, k, :],
                                    start=(k == 0), stop=(k == 1))
                            res = agsb.tile([P, H], bfl, tag="res")
                            nc.vector.tensor_copy(res[:], pz[:])
                        else:
                            res = hb
                        out_fn(t, res)

            def z1_write(t, res):
                if t < TS1:
                    nc.sync.dma_start(z1a[t * P:(t + 1) * P, :], res[:])
                else:
                    t2 = t - TS1
                    nc.sync.dma_start(z1b[t2 * P:(t2 + 1) * P, :], res[:])

            def l1_hook(t):
                if t == TS1 + 3:
                    nc.gpsimd.collective_compute(
                        "AllGather", mybir.AluOpType.bypass,
                        replica_groups=rg, ins=[z1a[:]], outs=[Z1a[:]])

            agg_layer(Z0a[:], Z0b[:], T1, cl1, ch1, gidx1_sb, stt1, bb1_sb,
                      z1_write, do_conv2=True, hook=l1_hook)
            nc.gpsimd.collective_compute(
                "AllGather", mybir.AluOpType.bypass, replica_groups=rg,
                ins=[z1b[:]], outs=[Z1b[:]])

            def h2_write(t, res):
                nc.sync.dma_start(h2loc[t * P:(t + 1) * P, :], res[:])

            agg_layer(Z1a[:], Z1b[:], T2, cl2, ch2, gidx2_sb, stt2, bb2_sb,
                      h2_write, do_conv2=False)

            # ---------------- head (owner-local variants)
            with tc.tile_pool(name="hdsb", bufs=2) as hdsb, \
                 tc.tile_pool(name="hdps", bufs=1, space="PSUM") as hdps:
                g = hdsb.tile([P, BCH, H], bfl, tag="hg")
                nc.gpsimd.dma_gather(
                    g[:], h2loc[:], hidx_sb[:], CAP, CAP, H,
                    single_packet=False)
                zt0 = hdsb.tile([P, CAP], bfl, tag="zt0")
                zt1 = hdsb.tile([P, CAP], bfl, tag="zt1")
                for j in range(BCH):
                    for k in range(2):
                        pt = hdps.tile([P, P], bfl, space="PSUM", tag="hpt")
                        nc.tensor.transpose(
                            pt[:], g[:, j, k * P:(k + 1) * P], ident[:])
                        dstt = zt0 if k == 0 else zt1
                        nc.vector.tensor_copy(
                            dstt[:, j * P:(j + 1) * P], pt[:])
                ph1 = hdps.tile([P, CAP], f32, tag="ph1")
                for c0 in range(0, CAP, 512):
                    cw = min(512, CAP - c0)
                    nc.tensor.matmul(ph1[:, c0:c0 + cw],
                                     lhsT=wh1_sb[:, 0, :],
                                     rhs=zt0[:, c0:c0 + cw],
                                     start=True, stop=False)
                    nc.tensor.matmul(ph1[:, c0:c0 + cw],
                                     lhsT=wh1_sb[:, 1, :],
                                     rhs=zt1[:, c0:c0 + cw],
                                     start=False, stop=False)
                    nc.tensor.matmul(ph1[:, c0:c0 + cw],
                                     lhsT=wh1_sb[:OH, 2, :],
                                     rhs=ohT_sb[:, c0:c0 + cw],
                                     start=False, stop=True)
                a1 = hdsb.tile([P, CAP], bfl, tag="a1")
                nc.scalar.activation(a1[:], ph1[:],
                                     mybir.ActivationFunctionType.Relu,
                                     bias=bh1_sb[:])
                ph2 = hdps.tile([HH // 2, CAP], f32, tag="ph2")
                for c0 in range(0, CAP, 512):
                    cw = min(512, CAP - c0)
                    nc.tensor.matmul(ph2[:, c0:c0 + cw], lhsT=wh2_sb[:],
                                     rhs=a1[:, c0:c0 + cw],
                                     start=True, stop=True)
                a2 = hdsb.tile([HH // 2, CAP], bfl, tag="a2")
                nc.scalar.activation(a2[:], ph2[:],
                                     mybir.ActivationFunctionType.Relu,
                                     bias=bh2_sb[:])
                ph3 = hdps.tile([1, CAP], f32, tag="ph3")
                for c0 in range(0, CAP, 512):
                    cw = min(512, CAP - c0)
                    nc.tensor.matmul(ph3[:, c0:c0 + cw], lhsT=wh3_sb[:],
                                     rhs=a2[:, c0:c0 + cw],
                                     start=True, stop=True)
                osb = hdsb.tile([1, CAP], f32, tag="osb")
                nc.vector.tensor_scalar_add(osb[:], ph3[:], bh3_sb[:, :1])
                nc.sync.dma_start(out[:], osb[:])

    nc.compile()
    return nc


# ------------------------------------------------------------------ driver

_CACHE = {}


def _get_program(cfg, meta):
    key = (cfg.N, cfg.E, cfg.D_IN, cfg.B) + tuple(sorted(meta.items()))
    if key not in _CACHE:
        _CACHE[key] = build_program(cfg, meta)
    return _CACHE[key]


def kernel(**inputs):
    cfg = REAL
    in_maps, meta, out_pos = host_prep(cfg, **inputs)
    nc = _get_program(cfg, meta)
    from concourse import bass_utils
    res = bass_utils.run_bass_kernel_spmd(
        nc, in_maps, core_ids=list(range(NCORES)))
    full = np.zeros(cfg.B, np.float32)
    for q in range(NCORES):
        vals = np.asarray(res.results[q]["out"]).reshape(-1)
        full[out_pos[q]] = vals[:len(out_pos[q])]
    return full.astype(np.float32)
